# revision 13
# baseline (speedup 1.0000x reference)
"""Paged GQA attention (diffusion-LM, bidirectional) on 8 Trainium2 NeuronCores.

Sharding: sequence s -> core s (8 sequences, 8 cores). Each core computes full
attention for one sequence: 32 q heads (8 KV heads x GQA group 4), q_len 64,
context 2048 cached tokens (gathered per block table) + 64 new tokens.

Per-core device kernel (all matmuls bf16, accumulation f32):
  scores^T[tok, g*q] = K^T_chunk.T @ Q^T   (lhsT = K^T chunk [128d,128tok])
  P = exp(scores^T)                         (ScalarE, no max-subtraction:
                                             scores ~ N(0,1), safe in f32)
  [O | denom] += P_chunk^T.T @ [V_chunk | 1] (ones column folds the
                                             partition-dim softmax sum into PE)
  out = O / denom                           (DVE reciprocal + scalar-mul)

K/V stream in group-sized pieces on the sync HWDGE ring in consumption order
(FIFO per ring), with tile-pool slot reuse providing just-in-time backpressure.
Output DMAs ride gpsimd SWDGE so their semaphore waits never stall the input
stream. Host side: block-table gather, head-major transposes, *scale folding
into q, f32->bf16 conversion (halves HBM traffic; abs-max rel err ~6e-3).
"""

import sys
import types

import numpy as np
import ml_dtypes

BF16 = ml_dtypes.bfloat16

# problem constants (hardcoded per spec)
S = 8            # sequences == cores
QL = 64          # active (new) tokens per sequence
NUM_HEADS = 32
HKV = 8          # kv heads
G = 4            # GQA group size
D = 128          # head dim
GQ = G * QL      # 256 q-rows per kv head
MEM_BLK = 32     # tokens per cache block
BLKS = 64        # blocks per sequence
CTX = MEM_BLK * BLKS          # 2048
T = CTX + QL                  # 2112 real tokens
NCH = 17                      # token chunks of 128 (64 tokens padding)
TP = NCH * 128                # 2176 padded tokens
GRPS = (6, 6, 5)              # chunks per exp batch (PSUM-bank sized)
SCALE = 0.08838834764831845

_CACHE = {}


def _install_ntff_hook():
    """bass_utils trace=True under axon needs antenv.axon_hooks; the staged
    antenv package lacks it, so synthesize the module and wire the ctypes
    NTFF hook from trn_agent_boot."""
    import antenv

    if "antenv.axon_hooks" not in sys.modules:
        mod = types.ModuleType("antenv.axon_hooks")
        holder = [None]
        mod.set_axon_ntff_profile_hook = lambda h: holder.__setitem__(0, h)
        mod.get_axon_ntff_profile_hook = lambda: holder[0]
        sys.modules["antenv.axon_hooks"] = mod
        antenv.axon_hooks = mod
    try:
        from trn_agent_boot.trn_boot import _ntff_profile_via_ctypes

        hook = _ntff_profile_via_ctypes("/opt/axon/libaxon_pjrt.so")
        if hook is not None:
            sys.modules["antenv.axon_hooks"].set_axon_ntff_profile_hook(hook)
    except Exception:
        pass


def _build_nc():
    if "nc" in _CACHE:
        return _CACHE["nc"]
    import concourse.bacc as bacc
    import concourse.tile as tile
    from concourse import mybir

    nc = bacc.Bacc("TRN2", target_bir_lowering=False, debug=False, num_devices=S)
    bf = mybir.dt.bfloat16
    f32 = mybir.dt.float32
    qt = nc.declare_dram_parameter("qt", [HKV, D, GQ], bf, isOutput=False)
    kt = nc.declare_dram_parameter("kt", [HKV, D, TP], bf, isOutput=False)
    va = nc.declare_dram_parameter("va", [HKV, 128, NCH, 129], bf, isOutput=False)
    out = nc.declare_dram_parameter("out", [HKV, GQ, D], f32, isOutput=True)

    goff = [0, 6, 12]  # first chunk of each group

    with tile.TileContext(nc) as tc:
        with (
            tc.tile_pool(name="qp", bufs=8) as q_pool,
            tc.tile_pool(name="kv", bufs=6) as kv_pool,
            tc.tile_pool(name="p", bufs=3) as p_pool,
            tc.tile_pool(name="qk", bufs=2, space="PSUM") as qk_pool,
            tc.tile_pool(name="ops", bufs=1, space="PSUM") as o_pool,
            tc.tile_pool(name="osb", bufs=4) as osb_pool,
        ):
            # Each HWDGE DIRECT2D costs ~0.6us of descriptor generation and
            # rings are FIFO, so keep issue counts low and split across both
            # rings: K (+q) on the sync ring, V on the scalar ring (its
            # sequencer is free while the ACT datapath crunches an EXP).
            # Head 0's K is split into group pieces so the first QK batch
            # starts as early as possible; later heads load whole-head.
            qt_sbs = []
            kt_sbs = {}   # (h, g) -> (tile, col offset)
            va_sbs = {}   # h -> (tile or [group tiles])

            def load_kt_group(h, g):
                gl = GRPS[g]
                c0 = goff[g]
                t = kv_pool.tile(
                    [128, gl * 128], bf, tag=f"kt{h}g{g}", name=f"kt_sb{h}_{g}", bufs=1
                )
                nc.sync.dma_start(out=t[:], in_=kt[h][:, c0 * 128 : (c0 + gl) * 128])
                kt_sbs[h, g] = (t, 0)

            def load_va_group(h, g):
                gl = GRPS[g]
                c0 = goff[g]
                # shared "va" tag: slot reuse chains every later V load behind
                # early PV progress, keeping V traffic out of the ramp
                t = kv_pool.tile(
                    [128, gl, 129], bf, tag="va", name=f"va_sb{h}_{g}", bufs=3,
                    padded_shape=[128, NCH, 129],
                )
                nc.sync.dma_start(out=t[:], in_=va[h][:, c0 : c0 + gl, :])
                return t

            def load_qt(h):
                t = q_pool.tile([128, GQ], bf, tag="qt", name=f"qt_sb{h}")
                nc.sync.dma_start(out=t[:], in_=qt[h])
                qt_sbs.append(t)

            # Heads 0-1: group-granular, need-ordered on the sync ring so the
            # pipeline ramps with minimal first-tile latency.
            for h in (0, 1):
                load_kt_group(h, 0)
                if h == 0:
                    load_qt(0)
                load_kt_group(h, 1)
                va_g = [load_va_group(h, 0)]
                load_kt_group(h, 2)
                va_g.append(load_va_group(h, 1))
                va_g.append(load_va_group(h, 2))
                va_sbs[h] = va_g
                if h == 0:
                    load_qt(1)
            # Heads 2+: whole-head; K (+q) + head-2 V on the sync ring, later
            # V on the scalar ring (shared "va" tag slots gate their start).
            for h in range(2, HKV):
                load_qt(h)
                kt_sb = kv_pool.tile(
                    [128, TP], bf, tag="kth", name=f"kt_sb{h}", bufs=3
                )
                nc.sync.dma_start(out=kt_sb[:], in_=kt[h])
                for g in range(len(GRPS)):
                    kt_sbs[h, g] = (kt_sb, goff[g] * 128)
                va_sb = kv_pool.tile(
                    [128, NCH, 129], bf, tag="va", name=f"va_sb{h}", bufs=3
                )
                eng = nc.sync if h == 2 else nc.scalar
                eng.dma_start(out=va_sb[:], in_=va[h])
                va_sbs[h] = va_sb

            # Software-pipelined emission over the 24 (head, group) units:
            # QK of unit i+1 is emitted BEFORE PV of unit i so the PE stream
            # never parks behind a PV that waits on the current EXP — keeps
            # ScalarE (the bottleneck) running back-to-back across heads.
            units = [(h, g) for h in range(HKV) for g in range(len(GRPS))]
            o_ps = {}
            p_tiles = {}
            qk_tiles = {}

            def emit_qk(i):
                h, g = units[i]
                gl = GRPS[g]
                qk = qk_pool.tile([128, gl * GQ], f32, tag="qk", name=f"qk{h}_{g}")
                kt_sb, off = kt_sbs[h, g]
                for cl in range(gl):
                    nc.tensor.matmul(
                        qk[:, cl * GQ : (cl + 1) * GQ],
                        lhsT=kt_sb[:, off + cl * 128 : off + (cl + 1) * 128],
                        rhs=qt_sbs[h][:],
                        start=True,
                        stop=True,
                    )
                qk_tiles[i] = qk

            def emit_exp(i):
                h, g = units[i]
                gl = GRPS[g]
                p_sb = p_pool.tile([128, gl * GQ], bf, tag="p", name=f"p_sb{h}_{g}")
                nc.scalar.activation(
                    p_sb[:], qk_tiles.pop(i)[:], mybir.ActivationFunctionType.Exp
                )
                p_tiles[i] = p_sb

            def emit_pv(i):
                h, g = units[i]
                gl = GRPS[g]
                if g == 0:
                    o_ps[h] = [
                        o_pool.tile(
                            [128, 129], f32, tag=f"o{half}", name=f"o_ps{h}_{half}"
                        )
                        for half in range(2)
                    ]
                p_sb = p_tiles.pop(i)
                va_t = va_sbs[h]
                for cl in range(gl):
                    c = goff[g] + cl
                    va_ap = (
                        va_t[g][:, cl, :] if isinstance(va_t, list)
                        else va_t[:, c, :]
                    )
                    for half in range(2):
                        nc.tensor.matmul(
                            o_ps[h][half][:],
                            lhsT=p_sb[
                                :, cl * GQ + half * 128 : cl * GQ + (half + 1) * 128
                            ],
                            rhs=va_ap,
                            start=(c == 0),
                            stop=(c == NCH - 1),
                        )
                if g == len(GRPS) - 1:
                    emit_out(h)

            def emit_out(h):
                o_sb = osb_pool.tile([128, 2, D], f32, tag="osb", name=f"o_sb{h}")
                for half in range(2):
                    recip = osb_pool.tile(
                        [128, 1], f32, tag="recip", name=f"recip{h}_{half}"
                    )
                    nc.vector.reciprocal(recip[:], o_ps[h][half][:, 128:129])
                    nc.vector.tensor_scalar_mul(
                        o_sb[:, half, :], o_ps[h][half][:, 0:D], recip[:]
                    )
                # one DMA per head; late heads ride the (by then idle) sync
                # HWDGE ring: ~0.6us latency vs ~2us SWDGE, shorter tail.
                eng = nc.sync if h >= HKV - 2 else nc.gpsimd
                eng.dma_start(
                    out=out[h].rearrange("(a p) d -> p a d", a=2), in_=o_sb[:]
                )

            emit_qk(0)
            for i in range(len(units)):
                if i + 1 < len(units):
                    emit_qk(i + 1)
                emit_exp(i)
                emit_pv(i)
    nc.compile()
    _CACHE["nc"] = nc
    return nc


def _shard_inputs(q, k, v, k_cache, v_cache, block_tables):
    """Build per-core input maps (host-side gather + layout + bf16)."""
    in_maps = []
    for s in range(S):
        # Q: [64, 4096] -> [h, d, g*q], scale folded in
        qs = q[s * QL : (s + 1) * QL].reshape(QL, HKV, G, D)
        qt = (qs.transpose(1, 3, 2, 0).reshape(HKV, D, GQ) * SCALE).astype(BF16)

        # K: gather ctx blocks + new tokens -> [T, HKV, D], pad, transpose
        kc = k_cache[block_tables[s]].reshape(CTX, HKV, D)
        kn = k[s * QL : (s + 1) * QL].reshape(QL, HKV, D)
        kf = np.zeros((TP, HKV, D), dtype=np.float32)
        kf[:CTX] = kc
        kf[CTX:T] = kn
        kt = np.ascontiguousarray(kf.transpose(1, 2, 0)).astype(BF16)  # [h, d, tp]

        # V + ones column (zero on padding) -> [h, part, chunk, 129]
        vc = v_cache[block_tables[s]].reshape(CTX, HKV, D)
        vn = v[s * QL : (s + 1) * QL].reshape(QL, HKV, D)
        vf = np.zeros((TP, HKV, D + 1), dtype=np.float32)
        vf[:CTX, :, :D] = vc
        vf[CTX:T, :, :D] = vn
        vf[:T, :, D] = 1.0
        # token t = c*128 + p  ->  va[h, p, c, :]
        va = np.ascontiguousarray(
            vf.reshape(NCH, 128, HKV, D + 1).transpose(2, 1, 0, 3)
        ).astype(BF16)

        in_maps.append({"qt": qt, "kt": kt, "va": va})
    return in_maps


def _unshard_output(results):
    """Per-core out [HKV, GQ, D] f32 -> full [S*QL, NUM_HEADS*D]."""
    full = np.empty((S * QL, NUM_HEADS * D), dtype=np.float32)
    for s in range(S):
        o = results[s]["out"].reshape(HKV, G, QL, D)
        full[s * QL : (s + 1) * QL] = (
            o.transpose(2, 0, 1, 3).reshape(QL, NUM_HEADS * D)
        )
    return full


def _run(inputs, trace=False):
    from concourse.bass_utils import run_bass_kernel_spmd

    if trace:
        _install_ntff_hook()
    nc = _build_nc()
    in_maps = _shard_inputs(**inputs)
    res = run_bass_kernel_spmd(nc, in_maps, core_ids=list(range(S)), trace=trace)
    return _unshard_output(res.results), res


def kernel(q, k, v, k_cache, v_cache, block_tables):
    out, _ = _run(
        dict(q=q, k=k, v=v, k_cache=k_cache, v_cache=v_cache, block_tables=block_tables)
    )
    return out
